# revision 8
# baseline (speedup 1.0000x reference)
"""Trainium2 Bass kernel for CrossAttention (B=4, QL=KL=2048, D=1024, fp32).

reference:
    query = hidden_states @ Wq                      # [B, QL, D]
    kv    = decoder_hidden_states @ Wkv             # [B, KL, 2D]
    key, value = split(kv, 2, axis=-1)
    scores = einsum('bqd,bkd->bqk', query, key) / sqrt(D)
    w = softmax(scores, axis=-1)
    out = einsum('bqk,bkd->bqd', w, value)          # [B, QL, D]

Sharding: 8 cores = batch(4) x q-half(2).  Each core owns 1024 query rows of
one batch and computes the full K/V projection for its batch (KV work
duplicated x2 across the pair sharing a batch; no collectives needed).

All matmuls run in float32r (TF32-like), which streams at full PE rate for
moving dims >= 256.  Softmax runs without max-subtraction (scores here are
~N(0,1); exp stays far from fp32 limits) using ACT's fused exp(scale*x) with
accum_out row sums.

This walrus build allows only ONE embedded semaphore wait per hardware
instruction; legalize_waits() splits any extra waits onto injected
same-engine NOPs after Tile scheduling.
"""

import sys

if "/opt/trn_rl_repo" not in sys.path:
    sys.path.insert(0, "/opt/trn_rl_repo")

import numpy as np

import bass_rust
import concourse.bass as bass
import concourse.mybir as mybir
import concourse.tile as tile
from concourse.bass_utils import run_bass_kernel_spmd

F32 = mybir.dt.float32
F32R = mybir.dt.float32r
EXP = mybir.ActivationFunctionType.Exp

N_CORES = 8
B, QL, KL, D = 4, 2048, 2048, 1024
CW = 256  # streaming chunk width for projection rhs (SBUF budget)


def legalize_waits(nc, max_waits=1):
    """TRN2 instructions embed at most one semaphore wait.  Move excess waits
    emitted by Tile onto same-engine NOPs inserted just before the owning
    instruction (engine FIFO makes this semantically identical)."""
    cnt = 0
    for fn in nc.m.functions:
        for bb in fn.blocks:
            out = []
            changed = False
            for ins in bb.instructions:
                si = ins.sync_info
                if si is not None and si.on_wait and len(si.on_wait) > max_waits:
                    waits = list(si.on_wait)
                    for w in waits[:-max_waits]:
                        cnt += 1
                        nop = bass_rust.InstNoOp(name=f"I-wfix-{cnt}")
                        nop.engine = ins.engine
                        nop.sync_info = mybir.SyncInfo(on_wait=[w], on_update=[])
                        out.append(nop)
                    ins.sync_info = mybir.SyncInfo(
                        on_wait=waits[-max_waits:],
                        on_update=list(si.on_update or []),
                    )
                    changed = True
                out.append(ins)
            if changed:
                bb.instructions = out
    return cnt


def build_attention(nc, QS, KLp, Dp, scale):
    DS = Dp // 128          # contraction subtiles
    NDO = Dp // 128         # output-d 128-chunks
    NKC = KLp // 512        # k 512-chunks (scores)
    NKT = KLp // 128        # k 128-chunks
    NQT = QS // 128         # q tiles
    NDC = Dp // 512         # d 512-chunks (AV)
    NAC = KLp // CW         # A1 rhs chunks
    NQC = QS // CW          # B rhs chunks

    hsT = nc.declare_dram_parameter("hsT", [128, DS, QS], F32R, isOutput=False)
    decT = nc.declare_dram_parameter("decT", [128, DS, KLp], F32R, isOutput=False)
    wq = nc.declare_dram_parameter("wq", [128, DS, Dp], F32R, isOutput=False)
    wkv = nc.declare_dram_parameter("wkv", [128, DS, 2 * Dp], F32R, isOutput=False)
    out = nc.declare_dram_parameter("out", [QS, Dp], F32, isOutput=True)

    with tile.TileContext(nc) as tc:
        with (
            tc.tile_pool(name="const", bufs=1) as constp,
            tc.tile_pool(name="ktv", bufs=1) as ktvp,
            tc.tile_pool(name="dram", bufs=1, space="DRAM") as dramp,
        ):
            ident = constp.tile([128, 128], F32)
            nc.gpsimd.memset(ident[:], 0.0)
            nc.gpsimd.affine_select(
                out=ident[:], in_=ident[:],
                compare_op=mybir.AluOpType.not_equal,
                fill=1.0, base=0, pattern=[[-1, 128]], channel_multiplier=1,
            )

            KT = ktvp.tile([128, DS, KLp], F32R, tag="KT")   # [d, k] rhs for scores
            V = ktvp.tile([128, NKT, Dp], F32R, tag="V")     # [k, d] rhs for AV
            qt_dram = dramp.tile([128, DS, QS], F32R)

            # ---------------- Phase A1: KT[do, k] = Wkv_lo^T @ decT -------------
            with (
                tc.tile_pool(name="wlo", bufs=NDO) as wlop,
                tc.tile_pool(name="dt1", bufs=2) as dt1p,
                tc.tile_pool(name="psA", bufs=3, space="PSUM") as psA,
            ):
                wlo = []
                for do in range(NDO):
                    t = wlop.tile([128, DS, 128], F32R, tag="wlo")
                    nc.sync.dma_start(t[:], wkv[:, :, do * 128 : (do + 1) * 128])
                    wlo.append(t)
                for kc in range(NAC):
                    dt = dt1p.tile([128, DS, CW], F32R, tag="dt1")
                    nc.sync.dma_start(dt[:], decT[:, :, kc * CW : (kc + 1) * CW])
                    for do in range(NDO):
                        ps = psA.tile([128, CW], F32, tag="psA")
                        for di in range(DS):
                            nc.tensor.matmul(
                                ps[:], wlo[do][:, di, :], dt[:, di, :],
                                start=(di == 0), stop=(di == DS - 1),
                            )
                        nc.vector.tensor_copy(
                            KT[:, do, kc * CW : (kc + 1) * CW], ps[:]
                        )

            # ---------------- Phase A2: V[k, d] = decT^T @ Wkv_hi ---------------
            with (
                tc.tile_pool(name="whi", bufs=2) as whip,
                tc.tile_pool(name="dt2", bufs=2) as dt2p,
                tc.tile_pool(name="psB", bufs=3, space="PSUM") as psB,
            ):
                whis = []
                for dc in range(NDC):
                    w = whip.tile([128, DS, 512], F32R, tag="whi", name=f"whi{dc}")
                    nc.sync.dma_start(w[:], wkv[:, :, Dp + dc * 512 : Dp + (dc + 1) * 512])
                    whis.append(w)
                for kt in range(NKT):
                    dt = dt2p.tile([128, DS, 128], F32R, tag="dt2")
                    nc.sync.dma_start(dt[:], decT[:, :, kt * 128 : (kt + 1) * 128])
                    for dc in range(NDC):
                        ps = psB.tile([128, 512], F32, tag="psB")
                        for di in range(DS):
                            nc.tensor.matmul(
                                ps[:], dt[:, di, :], whis[dc][:, di, :],
                                start=(di == 0), stop=(di == DS - 1),
                            )
                        nc.vector.tensor_copy(
                            V[:, kt, dc * 512 : (dc + 1) * 512], ps[:]
                        )

            # ---------------- Phase B: QT[do, q] = Wq^T @ hsT -> DRAM -----------
            with (
                tc.tile_pool(name="wqp", bufs=NDO) as wqp,
                tc.tile_pool(name="hst", bufs=2) as hstp,
                tc.tile_pool(name="stg", bufs=4) as stgp,
                tc.tile_pool(name="psC", bufs=3, space="PSUM") as psC,
            ):
                wqt = []
                for do in range(NDO):
                    t = wqp.tile([128, DS, 128], F32R, tag="wqp")
                    nc.sync.dma_start(t[:], wq[:, :, do * 128 : (do + 1) * 128])
                    wqt.append(t)
                for qc in range(NQC):
                    ht = hstp.tile([128, DS, CW], F32R, tag="hst")
                    nc.sync.dma_start(ht[:], hsT[:, :, qc * CW : (qc + 1) * CW])
                    for do in range(NDO):
                        ps = psC.tile([128, CW], F32, tag="psC")
                        for di in range(DS):
                            nc.tensor.matmul(
                                ps[:], wqt[do][:, di, :], ht[:, di, :],
                                start=(di == 0), stop=(di == DS - 1),
                            )
                        st = stgp.tile([128, CW], F32R, tag="stg")
                        nc.vector.tensor_copy(st[:], ps[:])
                        nc.sync.dma_start(
                            qt_dram[:, do, qc * CW : (qc + 1) * CW], st[:]
                        )

            # ---------------- Phase C: attention per q-tile ---------------------
            with (
                tc.tile_pool(name="qt", bufs=3) as qtp,
                tc.tile_pool(name="pp", bufs=2) as pp,
                tc.tile_pool(name="ptp", bufs=2) as ptp,
                tc.tile_pool(name="stat", bufs=NQT) as statp,
                tc.tile_pool(name="ost", bufs=2) as ostp,
                tc.tile_pool(name="ps_sc", bufs=3, space="PSUM") as ps_sc,
                tc.tile_pool(name="ps_pt", bufs=2, space="PSUM") as ps_pt,
                tc.tile_pool(name="ps_av", bufs=2, space="PSUM") as ps_av,
            ):
                for qt in range(NQT):
                    qtile = qtp.tile([128, DS, 128], F32R, tag="qt")
                    nc.sync.dma_start(qtile[:], qt_dram[:, :, qt * 128 : (qt + 1) * 128])

                    P = pp.tile([128, KLp], F32R, tag="pp")
                    lpart = statp.tile([128, NKC + 1], F32, tag="stat")
                    for kc in range(NKC):
                        ps = ps_sc.tile([128, 512], F32, tag="ps_sc")
                        for di in range(DS):
                            nc.tensor.matmul(
                                ps[:], qtile[:, di, :],
                                KT[:, di, kc * 512 : (kc + 1) * 512],
                                start=(di == 0), stop=(di == DS - 1),
                            )
                        nc.scalar.activation(
                            P[:, kc * 512 : (kc + 1) * 512], ps[:], EXP,
                            bias=0.0, scale=float(scale),
                            accum_out=lpart[:, kc : kc + 1],
                        )
                    nc.vector.tensor_tensor(
                        lpart[:, NKC : NKC + 1], lpart[:, 0:1], lpart[:, 1:2],
                        mybir.AluOpType.add,
                    )
                    for kc in range(2, NKC):
                        nc.vector.tensor_tensor(
                            lpart[:, NKC : NKC + 1], lpart[:, NKC : NKC + 1],
                            lpart[:, kc : kc + 1], mybir.AluOpType.add,
                        )
                    recip = statp.tile([128, 1], F32, tag="recip")
                    nc.vector.reciprocal(recip[:], lpart[:, NKC : NKC + 1])

                    PT = ptp.tile([128, NKT, 128], F32R, tag="ptp")
                    avs = [
                        ps_av.tile([128, 512], F32, tag="ps_av", name=f"av{i}")
                        for i in range(NDC)
                    ]
                    for kt in range(NKT):
                        pst = ps_pt.tile([128, 128], F32, tag="ps_pt")
                        nc.tensor.transpose(
                            pst[:], P[:, kt * 128 : (kt + 1) * 128].bitcast(F32),
                            ident[:],
                        )
                        nc.vector.tensor_copy(PT[:, kt, :], pst[:])
                        for dc in range(NDC):
                            nc.tensor.matmul(
                                avs[dc][:], PT[:, kt, :],
                                V[:, kt, dc * 512 : (dc + 1) * 512],
                                start=(kt == 0), stop=(kt == NKT - 1),
                            )
                    ot = ostp.tile([128, Dp], F32, tag="ost")
                    for dc in range(NDC):
                        nc.vector.tensor_scalar(
                            ot[:, dc * 512 : (dc + 1) * 512], avs[dc][:],
                            recip[:], None, mybir.AluOpType.mult,
                        )
                    nc.sync.dma_start(out[qt * 128 : (qt + 1) * 128, :], ot[:])

    legalize_waits(nc)
    return nc


def _pack_dT(x):
    """[N, Dp] row-major -> [128, Dp//128, N] with d on partitions:
    result[p, s, n] = x[n, s*128 + p]."""
    Dp = x.shape[1]
    return np.ascontiguousarray(x.T.reshape(Dp // 128, 128, -1).transpose(1, 0, 2))


def kernel(hidden_states, decoder_hidden_states, Wq, Wkv):
    hidden_states = np.asarray(hidden_states, dtype=np.float32)
    decoder_hidden_states = np.asarray(decoder_hidden_states, dtype=np.float32)
    Wq = np.asarray(Wq, dtype=np.float32)
    Wkv = np.asarray(Wkv, dtype=np.float32)

    QS = QL // 2
    scale = 1.0 / float(np.sqrt(D))

    nc = bass.Bass()
    build_attention(nc, QS, KL, D, scale)

    # Wq is [D_in, D_out]; lhsT layout needs wq[p, s, o] = Wq[s*128+p, o]
    wq_p = _pack_dT(Wq.T)
    wkv_p = _pack_dT(Wkv.T)

    in_maps = []
    for c in range(N_CORES):
        b, h = c // 2, c % 2
        hs = hidden_states[b, h * QS : (h + 1) * QS]        # [QS, D]
        dec = decoder_hidden_states[b]                      # [KL, D]
        in_maps.append(
            {
                "hsT": _pack_dT(hs),     # [128, DS, QS]
                "decT": _pack_dT(dec),   # [128, DS, KL]
                "wq": wq_p,
                "wkv": wkv_p,
            }
        )

    res = run_bass_kernel_spmd(nc, in_maps, list(range(N_CORES)))

    out = np.empty((B, QL, D), dtype=np.float32)
    for c in range(N_CORES):
        b, h = c // 2, c % 2
        out[b, h * QS : (h + 1) * QS] = res.results[c]["out"]
    return out
